# revision 11
# baseline (speedup 1.0000x reference)
"""Trainium2 Bass kernel for a 3-layer GAT (nn_GAT_30030411334390).

Strategy
--------
* Shard by destination node range: core c owns dst nodes [c*6250, (c+1)*6250).
  Each core aggregates messages for its own dst nodes only -> no reduce
  collectives are needed, just an AllGather of the per-node feature table
  between layers.
* Attention math is restructured so no per-edge transcendentals are needed:
      exp(leakyrelu(s1+s2)) = max(e^(s1+s2), e^(0.2(s1+s2)))
                            = max(u1[src]*p[dst], u1h[src]*ph[dst])
  with u1 = exp(s1), u1h = exp(0.2*s1) per node (and p/ph from s2).
  The segment-max subtraction of the reference softmax cancels exactly.
* Layer 1 aggregates x (256 wide) BEFORE the GEMM (linearity); layers 2/3
  aggregate post-GEMM features (256 / 40 wide).
* Per dst-tile of 128 nodes, all in-edges are gathered with one dma_gather
  per src half-table (int16 index limit), rows [feat | u1 | u1h | 1 | pad]
  in bf16.  Edge->dst selection matrices (static, graph-only) are built on
  the host and streamed; each 128-edge chunk costs one DVE op (weight
  scaling) and one accumulating PE matmul.
"""

import os
import sys

import numpy as np
import ml_dtypes

sys.path.insert(0, "/opt/trn_rl_repo")

import concourse.bass as bass
from concourse import bacc
import concourse.mybir as mybir
import concourse.tile as tile
from concourse.bass_utils import run_bass_kernel_spmd

BF16 = ml_dtypes.bfloat16
AF = mybir.ActivationFunctionType
ALU = mybir.AluOpType
AX = mybir.AxisListType


class Cfg:
    N = 50000          # nodes
    E = 800000         # edges
    C = 8              # cores
    P = 128
    FIN = 256          # x width
    F1 = 512           # layer-1 GEMM output width
    F2 = 256           # layer-2 feature width
    F3 = 40            # n classes
    ND = N // C        # dst nodes per core
    T = (ND + P - 1) // P      # dst tiles per core
    HALF = 25000       # src half-table size (< 32768 for int16 gather idx)
    # gather-table row sizes in bf16 elements (multiple of 128 for dma_gather)
    ELEM = (384, 384, 128)
    # aggregated feature widths per layer
    FAGG = (256, 256, 40)
    K_half = None      # chunks of 128 edges per (tile, half); set from data


def _wrap_idx(idx_rows: np.ndarray) -> np.ndarray:
    """[G, Kc] int16 -> [G, 128, Kc//16] in dma_gather SBUF layout:
    element j of a gather goes to partition j%16, column j//16, and the
    16-partition pattern is replicated 8x down the 128 partitions."""
    G, Kc = idx_rows.shape
    w = idx_rows.reshape(G, Kc // 16, 16).transpose(0, 2, 1)  # [G,16,W]
    return np.tile(w, (1, 8, 1))  # [G,128,W]


def preprocess(x, edge_idx, W1, a1s, a1d, W2, a2s, a2d, W3, a3s, a3d):
    """Host-side sharding / metadata construction. Returns (in_maps, cfg)."""
    cfg = Cfg()
    N, E, C, P, T = cfg.N, cfg.E, cfg.C, cfg.P, cfg.T
    x = np.asarray(x, dtype=np.float32)
    src = np.asarray(edge_idx[0], dtype=np.int64)
    dst = np.asarray(edge_idx[1], dtype=np.int64)

    # ---- group edges by (core, dst-tile, src-half) ----
    core = dst // cfg.ND
    rel = dst - core * cfg.ND
    tl = rel // P
    ld = rel - tl * P
    half = (src >= cfg.HALF).astype(np.int64)
    gid = ((core * T + tl) * 2 + half).astype(np.int64)
    NG = C * T * 2
    order = np.argsort(gid, kind="stable")
    counts = np.bincount(gid, minlength=NG)
    offs = np.zeros(NG + 1, dtype=np.int64)
    np.cumsum(counts, out=offs[1:])
    pos = np.arange(E, dtype=np.int64) - offs[gid[order]]

    K_half = int((counts.max() + P - 1) // P)
    cfg.K_half = K_half
    Kc = K_half * P
    CH = 2 * K_half

    # ---- int16 gather indices (pads -> 0, masked via zero rows in Sel) ----
    idx_rows = np.zeros((NG, Kc), dtype=np.int16)
    idx_rows[gid[order], pos] = (src[order] - cfg.HALF * half[order]).astype(np.int16)
    idx_wrapped = _wrap_idx(idx_rows).reshape(C, T, 2, 128, Kc // 16)
    # halves side by side on the free dim: [C, T, 128, 2*W]
    idxs = idx_wrapped.transpose(0, 1, 3, 2, 4).reshape(C, T, 128, 2 * (Kc // 16))
    idxs = np.ascontiguousarray(idxs)

    # ---- selection matrices (static, shared across layers) ----
    # Sel[c,t]  [128e, CH*128]: chunk k block has [e, d]=1 iff edge slot e of
    #   chunk k targets local dst d.  Pad slots are zero rows.
    # SelT[c,t] [128d, CH*128]: transposed blocks.
    sel = np.zeros((C, T, 128, CH * 128), dtype=BF16)
    selt = np.zeros((C, T, 128, CH * 128), dtype=BF16)
    e_core = core[order]
    e_tile = tl[order]
    e_ld = ld[order]
    e_half = half[order]
    chunk = e_half * K_half + pos // P
    eslot = pos % P
    sel[e_core, e_tile, eslot, chunk * 128 + e_ld] = 1
    selt[e_core, e_tile, e_ld, chunk * 128 + eslot] = 1

    # ---- layer-1 node scalars (host: tiny matvecs on inputs) ----
    b1s = (np.asarray(W1) @ np.asarray(a1s)).astype(np.float32)
    b1d = (np.asarray(W1) @ np.asarray(a1d)).astype(np.float32)
    s1 = x @ b1s
    s2 = x @ b1d

    table1 = np.zeros((N, cfg.ELEM[0]), dtype=BF16)
    table1[:, : cfg.FIN] = x.astype(BF16)
    table1[:, cfg.FIN] = np.exp(s1).astype(BF16)
    table1[:, cfg.FIN + 1] = np.exp(0.2 * s1).astype(BF16)
    table1[:, cfg.FIN + 2] = 1.0

    # per-core per-dst p/ph for layer 1: [128, 2*T]
    p1 = np.zeros((C, 128, 2 * T), dtype=BF16)
    s2_pad = np.zeros(C * T * P, dtype=np.float32)
    for c in range(C):
        s2_pad[c * T * P : c * T * P + cfg.ND] = s2[c * cfg.ND : (c + 1) * cfg.ND]
    s2g = s2_pad.reshape(C, T, P)
    p1[:, :, 0::2] = np.exp(s2g).transpose(0, 2, 1).astype(BF16)
    p1[:, :, 1::2] = np.exp(0.2 * s2g).transpose(0, 2, 1).astype(BF16)

    # ---- weights (bf16, augmented with attention columns) ----
    w1 = np.asarray(W1, dtype=np.float32).astype(BF16)  # [256, 512]
    w2e = np.concatenate(
        [
            np.asarray(W2, dtype=np.float32),
            (np.asarray(W2) @ np.asarray(a2s))[:, None],
            (np.asarray(W2) @ np.asarray(a2d))[:, None],
        ],
        axis=1,
    ).astype(BF16)  # [512, 258]
    w3e = np.concatenate(
        [
            np.asarray(W3, dtype=np.float32),
            (np.asarray(W3) @ np.asarray(a3s))[:, None],
            (np.asarray(W3) @ np.asarray(a3d))[:, None],
        ],
        axis=1,
    ).astype(BF16)  # [256, 42]

    in_maps = []
    for c in range(C):
        in_maps.append(
            {
                "table1": table1,
                "sel": np.ascontiguousarray(sel[c]),
                "selt": np.ascontiguousarray(selt[c]),
                "idxs": np.ascontiguousarray(idxs[c]),
                "p1": np.ascontiguousarray(p1[c]),
                "w1": w1,
                "w2e": w2e,
                "w3e": w3e,
            }
        )
    return in_maps, cfg


def build_program(cfg):
    N, C, P, T = cfg.N, cfg.C, cfg.P, cfg.T
    K_half = cfg.K_half
    Kc = K_half * P
    CH = 2 * K_half
    W = Kc // 16
    bf = mybir.dt.bfloat16
    f32 = mybir.dt.float32
    i16 = mybir.dt.int16

    nc = bacc.Bacc("TRN2", num_devices=C)

    table1 = nc.dram_tensor("table1", [N, cfg.ELEM[0]], bf, kind="ExternalInput")
    sel_in = nc.dram_tensor("sel", [T, 128, CH * 128], bf, kind="ExternalInput")
    selt_in = nc.dram_tensor("selt", [T, 128, CH * 128], bf, kind="ExternalInput")
    idxs_in = nc.dram_tensor("idxs", [T, 128, 2 * W], i16, kind="ExternalInput")
    p1_in = nc.dram_tensor("p1", [128, 2 * T], bf, kind="ExternalInput")
    w1_in = nc.dram_tensor("w1", [256, 512], bf, kind="ExternalInput")
    w2e_in = nc.dram_tensor("w2e", [512, 258], bf, kind="ExternalInput")
    w3e_in = nc.dram_tensor("w3e", [256, 42], bf, kind="ExternalInput")
    out_d = nc.dram_tensor("out", [cfg.ND, cfg.F3], f32, kind="ExternalOutput")

    agin2 = nc.dram_tensor("agin2", [cfg.ND, cfg.ELEM[1]], bf)
    table2 = nc.dram_tensor("table2", [N, cfg.ELEM[1]], bf, addr_space="Shared")
    agin3 = nc.dram_tensor("agin3", [cfg.ND, cfg.ELEM[2]], bf)
    table3 = nc.dram_tensor("table3", [N, cfg.ELEM[2]], bf, addr_space="Shared")
    tables = (table1, table2, table3)

    with tile.TileContext(nc) as tc:
        with (
            tc.tile_pool(name="const", bufs=1) as constp,
            tc.tile_pool(name="io", bufs=2) as iop,
            tc.tile_pool(name="g", bufs=2) as gp,
            tc.tile_pool(name="mw", bufs=3) as mwp,
            tc.tile_pool(name="small", bufs=2) as smp,
            tc.tile_pool(name="na", bufs=2) as nap,
            tc.tile_pool(name="psA", bufs=2, space="PSUM") as psA,
            tc.tile_pool(name="psG", bufs=1, space="PSUM") as psG,
            tc.tile_pool(name="psT", bufs=2, space="PSUM") as psT,
        ):
            # ---- persistent constants ----
            ident = constp.tile([128, 128], bf)
            from concourse.masks import make_identity

            make_identity(nc, ident[:])
            w1sb = constp.tile([128, 2 * 512], bf)
            for k in range(2):
                nc.sync.dma_start(
                    out=w1sb[:, k * 512 : (k + 1) * 512],
                    in_=w1_in[k * 128 : (k + 1) * 128, :],
                )
            w2esb = constp.tile([128, 4 * 258], bf)
            for k in range(4):
                nc.sync.dma_start(
                    out=w2esb[:, k * 258 : (k + 1) * 258],
                    in_=w2e_in[k * 128 : (k + 1) * 128, :],
                )
            w3esb = constp.tile([128, 2 * 42], bf)
            for k in range(2):
                nc.sync.dma_start(
                    out=w3esb[:, k * 42 : (k + 1) * 42],
                    in_=w3e_in[k * 128 : (k + 1) * 128, :],
                )
            p_sb = [
                constp.tile([128, 2 * T], bf, tag=f"p{l}", name=f"p_sb{l}")
                for l in range(3)
            ]
            nc.sync.dma_start(out=p_sb[0][:], in_=p1_in[:])
            kc_reg = nc.gpsimd.to_reg(Kc)

            for layer in range(3):
                elem = cfg.ELEM[layer]
                F = cfg.FAGG[layer]
                tbl = tables[layer]
                for t in range(T):
                    rows = min(P, cfg.ND - t * P)
                    # ---- streams ----
                    idx_t = iop.tile([128, 2 * W], i16, tag="idx")
                    nc.sync.dma_start(out=idx_t[:], in_=idxs_in[t])
                    sel_t = iop.tile([128, CH * 128], bf, tag="sel")
                    nc.sync.dma_start(out=sel_t[:], in_=sel_in[t])
                    selt_t = iop.tile([128, CH * 128], bf, tag="selt")
                    nc.sync.dma_start(out=selt_t[:], in_=selt_in[t])

                    # ---- gathers (one per src half) ----
                    g_t = gp.tile([128, CH * elem], bf, tag="g")
                    g3 = g_t[:].rearrange("p (c e) -> p c e", e=elem)
                    for h in range(2):
                        nc.gpsimd.dma_gather(
                            out_ap=g3[:, h * K_half : (h + 1) * K_half, :],
                            in_ap=tbl[h * cfg.HALF : (h + 1) * cfg.HALF, :],
                            idxs_ap=idx_t[:, h * W : (h + 1) * W],
                            num_idxs=Kc,
                            num_idxs_reg=kc_reg,
                            elem_size=elem,
                            single_packet=False,
                        )

                    # ---- per-edge p[dst]/ph[dst] via SelT matmuls ----
                    pl_ps = psA.tile([128, 2 * CH], f32, tag="pl")
                    for k in range(CH):
                        nc.tensor.matmul(
                            pl_ps[:, 2 * k : 2 * k + 2],
                            lhsT=selt_t[:, k * 128 : (k + 1) * 128],
                            rhs=p_sb[layer][:, 2 * t : 2 * t + 2],
                            start=True,
                            stop=True,
                        )
                    pl3 = pl_ps[:].rearrange("p (c two) -> p c two", two=2)

                    # ---- per-edge weights w = max(u1*p, u1h*ph) ----
                    # computed per src-half so each DVE op depends on only one
                    # gather DMA (ISA limit: 2 sync waits per instruction)
                    t1 = smp.tile([128, CH], f32, tag="t1")
                    t2 = smp.tile([128, CH], f32, tag="t2")
                    w_t = smp.tile([128, CH], f32, tag="w")
                    for h in range(2):
                        hs = slice(h * K_half, (h + 1) * K_half)
                        nc.vector.tensor_tensor(
                            out=t1[:, hs],
                            in0=g3[:, hs, F],
                            in1=pl3[:, hs, 0],
                            op=ALU.mult,
                        )
                        nc.vector.tensor_tensor(
                            out=t2[:, hs],
                            in0=g3[:, hs, F + 1],
                            in1=pl3[:, hs, 1],
                            op=ALU.mult,
                        )
                    nc.vector.tensor_tensor(
                        out=w_t[:], in0=t1[:], in1=t2[:], op=ALU.max
                    )

                    # ---- weighted aggregation into PSUM ----
                    agg_ps = psA.tile([128, F + 3], f32, tag="agg")
                    for k in range(CH):
                        mw = mwp.tile([128, 128], bf, tag="mw")
                        nc.vector.tensor_scalar(
                            out=mw[:],
                            in0=sel_t[:, k * 128 : (k + 1) * 128],
                            scalar1=w_t[:, k : k + 1],
                            scalar2=None,
                            op0=ALU.mult,
                        )
                        nc.tensor.matmul(
                            agg_ps[:],
                            lhsT=mw[:],
                            rhs=g3[:, k, 0 : F + 3],
                            start=(k == 0),
                            stop=(k == CH - 1),
                        )

                    # ---- normalize: out = agg / (den + 1e-9) ----
                    dtmp = smp.tile([128, 1], f32, tag="dtmp")
                    dinv = smp.tile([128, 1], f32, tag="dinv")
                    nc.vector.tensor_scalar(
                        out=dtmp[:],
                        in0=agg_ps[:, F + 2 : F + 3],
                        scalar1=1e-9,
                        scalar2=None,
                        op0=ALU.add,
                    )
                    nc.vector.reciprocal(dinv[:], dtmp[:])

                    if layer == 0:
                        na = nap.tile([128, 256], bf, tag="na")
                        nc.vector.tensor_scalar(
                            out=na[:],
                            in0=agg_ps[:, 0:256],
                            scalar1=dinv[:, 0:1],
                            scalar2=None,
                            op0=ALU.mult,
                        )
                        # transpose -> GEMM W1 -> relu -> transpose -> GEMM W2e
                        naT = nap.tile([128, 256], bf, tag="naT")
                        for fb in range(2):
                            trp = psT.tile([128, 128], bf, tag="tr")
                            nc.tensor.transpose(
                                trp[:], na[:, fb * 128 : (fb + 1) * 128], ident[:]
                            )
                            nc.scalar.copy(
                                naT[:, fb * 128 : (fb + 1) * 128], trp[:]
                            )
                        o1_ps = psG.tile([128, 512], f32, tag="gemm")
                        for k in range(2):
                            nc.tensor.matmul(
                                o1_ps[:],
                                lhsT=naT[:, k * 128 : (k + 1) * 128],
                                rhs=w1sb[:, k * 512 : (k + 1) * 512],
                                start=(k == 0),
                                stop=(k == 1),
                            )
                        r1 = nap.tile([128, 512], bf, tag="r1")
                        nc.scalar.activation(r1[:], o1_ps[:], AF.Relu)
                        r1T = nap.tile([128, 512], bf, tag="r1T")
                        for fb in range(4):
                            trp = psT.tile([128, 128], bf, tag="tr")
                            nc.tensor.transpose(
                                trp[:], r1[:, fb * 128 : (fb + 1) * 128], ident[:]
                            )
                            nc.scalar.copy(
                                r1T[:, fb * 128 : (fb + 1) * 128], trp[:]
                            )
                        h2_ps = psG.tile([128, 258], f32, tag="gemm2")
                        for k in range(4):
                            nc.tensor.matmul(
                                h2_ps[:],
                                lhsT=r1T[:, k * 128 : (k + 1) * 128],
                                rhs=w2esb[:, k * 258 : (k + 1) * 258],
                                start=(k == 0),
                                stop=(k == 3),
                            )
                        # table2 block: [h2 | u1 | u1h | 1]
                        blk = nap.tile([128, 259], bf, tag="blk")
                        nc.scalar.copy(blk[:, 0:256], h2_ps[:, 0:256])
                        nc.scalar.activation(
                            blk[:, 256:257], h2_ps[:, 256:257], AF.Exp
                        )
                        nc.scalar.activation(
                            blk[:, 257:258], h2_ps[:, 256:257], AF.Exp, scale=0.2
                        )
                        nc.vector.memset(blk[:, 258:259], 1.0)
                        nc.scalar.activation(
                            p_sb[1][:, 2 * t : 2 * t + 1], h2_ps[:, 257:258], AF.Exp
                        )
                        nc.scalar.activation(
                            p_sb[1][:, 2 * t + 1 : 2 * t + 2],
                            h2_ps[:, 257:258],
                            AF.Exp,
                            scale=0.2,
                        )
                        nc.sync.dma_start(
                            out=agin2[t * P : t * P + rows, 0:259],
                            in_=blk[:rows, :],
                        )
                    elif layer == 1:
                        # fused normalize+relu
                        r2 = nap.tile([128, 256], bf, tag="na")
                        nc.vector.tensor_scalar(
                            out=r2[:],
                            in0=agg_ps[:, 0:256],
                            scalar1=dinv[:, 0:1],
                            scalar2=0.0,
                            op0=ALU.mult,
                            op1=ALU.max,
                        )
                        r2T = nap.tile([128, 256], bf, tag="naT")
                        for fb in range(2):
                            trp = psT.tile([128, 128], bf, tag="tr")
                            nc.tensor.transpose(
                                trp[:], r2[:, fb * 128 : (fb + 1) * 128], ident[:]
                            )
                            nc.scalar.copy(
                                r2T[:, fb * 128 : (fb + 1) * 128], trp[:]
                            )
                        h3_ps = psG.tile([128, 42], f32, tag="gemm2")
                        for k in range(2):
                            nc.tensor.matmul(
                                h3_ps[:],
                                lhsT=r2T[:, k * 128 : (k + 1) * 128],
                                rhs=w3esb[:, k * 42 : (k + 1) * 42],
                                start=(k == 0),
                                stop=(k == 1),
                            )
                        blk = nap.tile([128, 43], bf, tag="blk3")
                        nc.scalar.copy(blk[:, 0:40], h3_ps[:, 0:40])
                        nc.scalar.activation(blk[:, 40:41], h3_ps[:, 40:41], AF.Exp)
                        nc.scalar.activation(
                            blk[:, 41:42], h3_ps[:, 40:41], AF.Exp, scale=0.2
                        )
                        nc.vector.memset(blk[:, 42:43], 1.0)
                        nc.scalar.activation(
                            p_sb[2][:, 2 * t : 2 * t + 1], h3_ps[:, 41:42], AF.Exp
                        )
                        nc.scalar.activation(
                            p_sb[2][:, 2 * t + 1 : 2 * t + 2],
                            h3_ps[:, 41:42],
                            AF.Exp,
                            scale=0.2,
                        )
                        nc.sync.dma_start(
                            out=agin3[t * P : t * P + rows, 0:43],
                            in_=blk[:rows, :],
                        )
                    else:
                        # softmax over the 40 classes
                        o3 = nap.tile([128, 40], f32, tag="o3")
                        nc.vector.tensor_scalar(
                            out=o3[:],
                            in0=agg_ps[:, 0:40],
                            scalar1=dinv[:, 0:1],
                            scalar2=None,
                            op0=ALU.mult,
                        )
                        m = smp.tile([128, 1], f32, tag="m")
                        nc.vector.reduce_max(out=m[:], in_=o3[:], axis=AX.X)
                        negm = smp.tile([128, 1], f32, tag="negm")
                        nc.vector.tensor_scalar(
                            out=negm[:],
                            in0=m[:],
                            scalar1=-1.0,
                            scalar2=None,
                            op0=ALU.mult,
                        )
                        e_t = nap.tile([128, 40], f32, tag="et")
                        nc.scalar.activation(
                            e_t[:], o3[:], AF.Exp, bias=negm[:, 0:1]
                        )
                        s = smp.tile([128, 1], f32, tag="s")
                        nc.vector.reduce_sum(out=s[:], in_=e_t[:], axis=AX.X)
                        sinv = smp.tile([128, 1], f32, tag="sinv")
                        nc.vector.reciprocal(sinv[:], s[:])
                        fin = nap.tile([128, 40], f32, tag="fin")
                        nc.vector.tensor_scalar(
                            out=fin[:],
                            in0=e_t[:],
                            scalar1=sinv[:, 0:1],
                            scalar2=None,
                            op0=ALU.mult,
                        )
                        nc.sync.dma_start(
                            out=out_d[t * P : t * P + rows, :], in_=fin[:rows, :]
                        )

                if layer == 0:
                    nc.gpsimd.collective_compute(
                        "AllGather",
                        ALU.bypass,
                        replica_groups=[list(range(C))],
                        ins=[agin2[:]],
                        outs=[table2[:]],
                    )
                elif layer == 1:
                    nc.gpsimd.collective_compute(
                        "AllGather",
                        ALU.bypass,
                        replica_groups=[list(range(C))],
                        ins=[agin3[:]],
                        outs=[table3[:]],
                    )
    nc.finalize()  # Bacc.compile(): wait-count legalization etc.
    return nc


def kernel(**inputs) -> np.ndarray:
    in_maps, cfg = preprocess(**inputs)
    nc = build_program(cfg)
    res = run_bass_kernel_spmd(nc, in_maps, core_ids=list(range(cfg.C)))
    outs = [res.results[c]["out"] for c in range(cfg.C)]
    return np.concatenate(outs, axis=0).astype(np.float32)


if __name__ == "__main__":
    import jax

    jax.config.update("jax_platforms", "cpu")
    import reference

    inputs = {k: np.asarray(v) for k, v in reference.setup_inputs().items()}
    out = kernel(**inputs)
    print("kernel output", out.shape, out.dtype)


# revision 13
# speedup vs baseline: 1.4078x; 1.4078x over previous
"""Trainium2 Bass kernel for a 3-layer GAT (nn_GAT_30030411334390).

Strategy
--------
* Shard by destination node range: core c owns dst nodes [c*6250, (c+1)*6250).
  Each core aggregates messages for its own dst nodes only -> no reduce
  collectives are needed, just an AllGather of the per-node feature table
  between layers.
* Attention math is restructured so no per-edge transcendentals are needed:
      exp(leakyrelu(s1+s2)) = max(e^(s1+s2), e^(0.2(s1+s2)))
                            = max(u1[src]*p[dst], u1h[src]*ph[dst])
  with u1 = exp(s1), u1h = exp(0.2*s1) per node (and p/ph from s2).
  The segment-max subtraction of the reference softmax cancels exactly.
* Layer 1 aggregates x (256 wide) BEFORE the GEMM (linearity); layers 2/3
  aggregate post-GEMM features (256 / 40 wide).
* Per dst-tile of 128 nodes, all in-edges are gathered with one dma_gather
  per src half-table (int16 index limit), rows [feat | u1 | u1h | 1 | pad]
  in bf16.  Edge->dst selection matrices (static, graph-only) are built on
  the host and streamed; each 128-edge chunk costs one DVE op (weight
  scaling) and one accumulating PE matmul.
"""

import os
import sys

import numpy as np
import ml_dtypes

sys.path.insert(0, "/opt/trn_rl_repo")

import concourse.bass as bass
from concourse import bacc
import concourse.mybir as mybir
import concourse.tile as tile
from concourse.bass_utils import run_bass_kernel_spmd

BF16 = ml_dtypes.bfloat16
AF = mybir.ActivationFunctionType
ALU = mybir.AluOpType
AX = mybir.AxisListType


class Cfg:
    N = 50000          # nodes
    E = 800000         # edges
    C = 8              # cores
    P = 128
    FIN = 256          # x width
    F1 = 512           # layer-1 GEMM output width
    F2 = 256           # layer-2 feature width
    F3 = 40            # n classes
    ND = N // C        # dst nodes per core
    T = (ND + P - 1) // P      # dst tiles per core
    HALF = 25000       # src half-table size (< 32768 for int16 gather idx)
    # gather-table row sizes in bf16 elements (multiple of 128 for dma_gather)
    ELEM = (384, 384, 128)
    # aggregated feature widths per layer
    FAGG = (256, 256, 40)
    K_half = None      # chunks of 128 edges per (tile, half); set from data


def _wrap_idx(idx_rows: np.ndarray) -> np.ndarray:
    """[G, Kc] int16 -> [G, 128, Kc//16] in dma_gather SBUF layout:
    element j of a gather goes to partition j%16, column j//16, and the
    16-partition pattern is replicated 8x down the 128 partitions."""
    G, Kc = idx_rows.shape
    w = idx_rows.reshape(G, Kc // 16, 16).transpose(0, 2, 1)  # [G,16,W]
    return np.tile(w, (1, 8, 1))  # [G,128,W]


def preprocess(x, edge_idx, W1, a1s, a1d, W2, a2s, a2d, W3, a3s, a3d):
    """Host-side sharding / metadata construction. Returns (in_maps, cfg)."""
    cfg = Cfg()
    N, E, C, P, T = cfg.N, cfg.E, cfg.C, cfg.P, cfg.T
    x = np.asarray(x, dtype=np.float32)
    src = np.asarray(edge_idx[0], dtype=np.int64)
    dst = np.asarray(edge_idx[1], dtype=np.int64)

    # ---- group edges by (core, dst-tile, src-half) ----
    core = dst // cfg.ND
    rel = dst - core * cfg.ND
    tl = rel // P
    ld = rel - tl * P
    half = (src >= cfg.HALF).astype(np.int64)
    gid = ((core * T + tl) * 2 + half).astype(np.int64)
    NG = C * T * 2
    order = np.argsort(gid, kind="stable")
    counts = np.bincount(gid, minlength=NG)
    offs = np.zeros(NG + 1, dtype=np.int64)
    np.cumsum(counts, out=offs[1:])
    pos = np.arange(E, dtype=np.int64) - offs[gid[order]]

    K_half = int((counts.max() + P - 1) // P)
    cfg.K_half = K_half
    Kc = K_half * P
    CH = 2 * K_half

    # ---- int16 gather indices (pads -> 0, masked via zero rows in Sel) ----
    idx_rows = np.zeros((NG, Kc), dtype=np.int16)
    idx_rows[gid[order], pos] = (src[order] - cfg.HALF * half[order]).astype(np.int16)
    idx_wrapped = _wrap_idx(idx_rows).reshape(C, T, 2, 128, Kc // 16)
    # halves side by side on the free dim: [C, T, 128, 2*W]
    idxs = idx_wrapped.transpose(0, 1, 3, 2, 4).reshape(C, T, 128, 2 * (Kc // 16))
    idxs = np.ascontiguousarray(idxs)

    # ---- selection matrices (static, shared across layers) ----
    # Sel[c,t]  [128e, CH*128]: chunk k block has [e, d]=1 iff edge slot e of
    #   chunk k targets local dst d.  Pad slots are zero rows.
    # SelT[c,t] [128d, CH*128]: transposed blocks.
    sel = np.zeros((C, T, 128, CH * 128), dtype=BF16)
    selt = np.zeros((C, T, 128, CH * 128), dtype=BF16)
    e_core = core[order]
    e_tile = tl[order]
    e_ld = ld[order]
    e_half = half[order]
    chunk = e_half * K_half + pos // P
    eslot = pos % P
    sel[e_core, e_tile, eslot, chunk * 128 + e_ld] = 1
    selt[e_core, e_tile, e_ld, chunk * 128 + eslot] = 1

    # ---- layer-1 node scalars (host: tiny matvecs on inputs) ----
    b1s = (np.asarray(W1) @ np.asarray(a1s)).astype(np.float32)
    b1d = (np.asarray(W1) @ np.asarray(a1d)).astype(np.float32)
    s1 = x @ b1s
    s2 = x @ b1d

    table1 = np.zeros((N, cfg.ELEM[0]), dtype=BF16)
    table1[:, : cfg.FIN] = x.astype(BF16)
    table1[:, cfg.FIN] = np.exp(s1).astype(BF16)
    table1[:, cfg.FIN + 1] = np.exp(0.2 * s1).astype(BF16)
    table1[:, cfg.FIN + 2] = 1.0

    # per-core per-dst p/ph for layer 1: [128, 2*T]
    p1 = np.zeros((C, 128, 2 * T), dtype=BF16)
    s2_pad = np.zeros(C * T * P, dtype=np.float32)
    for c in range(C):
        s2_pad[c * T * P : c * T * P + cfg.ND] = s2[c * cfg.ND : (c + 1) * cfg.ND]
    s2g = s2_pad.reshape(C, T, P)
    p1[:, :, 0::2] = np.exp(s2g).transpose(0, 2, 1).astype(BF16)
    p1[:, :, 1::2] = np.exp(0.2 * s2g).transpose(0, 2, 1).astype(BF16)

    # ---- weights (bf16, augmented with attention columns) ----
    w1 = np.asarray(W1, dtype=np.float32).astype(BF16)  # [256, 512]
    w2e = np.concatenate(
        [
            np.asarray(W2, dtype=np.float32),
            (np.asarray(W2) @ np.asarray(a2s))[:, None],
            (np.asarray(W2) @ np.asarray(a2d))[:, None],
        ],
        axis=1,
    ).astype(BF16)  # [512, 258]
    w3e = np.concatenate(
        [
            np.asarray(W3, dtype=np.float32),
            (np.asarray(W3) @ np.asarray(a3s))[:, None],
            (np.asarray(W3) @ np.asarray(a3d))[:, None],
        ],
        axis=1,
    ).astype(BF16)  # [256, 42]

    in_maps = []
    for c in range(C):
        in_maps.append(
            {
                "table1": table1,
                "sel": np.ascontiguousarray(sel[c]),
                "selt": np.ascontiguousarray(selt[c]),
                "idxs": np.ascontiguousarray(idxs[c]),
                "p1": np.ascontiguousarray(p1[c]),
                "w1": w1,
                "w2e": w2e,
                "w3e": w3e,
            }
        )
    return in_maps, cfg


def build_program(cfg):
    N, C, P, T = cfg.N, cfg.C, cfg.P, cfg.T
    K_half = cfg.K_half
    Kc = K_half * P
    CH = 2 * K_half
    W = Kc // 16
    bf = mybir.dt.bfloat16
    f32 = mybir.dt.float32
    i16 = mybir.dt.int16

    nc = bacc.Bacc("TRN2", num_devices=C, num_swdge_queues=4)

    table1 = nc.dram_tensor("table1", [N, cfg.ELEM[0]], bf, kind="ExternalInput")
    sel_in = nc.dram_tensor("sel", [T, 128, CH * 128], bf, kind="ExternalInput")
    selt_in = nc.dram_tensor("selt", [T, 128, CH * 128], bf, kind="ExternalInput")
    idxs_in = nc.dram_tensor("idxs", [T, 128, 2 * W], i16, kind="ExternalInput")
    p1_in = nc.dram_tensor("p1", [128, 2 * T], bf, kind="ExternalInput")
    w1_in = nc.dram_tensor("w1", [256, 512], bf, kind="ExternalInput")
    w2e_in = nc.dram_tensor("w2e", [512, 258], bf, kind="ExternalInput")
    w3e_in = nc.dram_tensor("w3e", [256, 42], bf, kind="ExternalInput")
    out_d = nc.dram_tensor("out", [cfg.ND, cfg.F3], f32, kind="ExternalOutput")

    agin2 = nc.dram_tensor("agin2", [cfg.ND, cfg.ELEM[1]], bf)
    table2 = nc.dram_tensor("table2", [N, cfg.ELEM[1]], bf, addr_space="Shared")
    agin3 = nc.dram_tensor("agin3", [cfg.ND, cfg.ELEM[2]], bf)
    table3 = nc.dram_tensor("table3", [N, cfg.ELEM[2]], bf, addr_space="Shared")
    tables = (table1, table2, table3)

    with tile.TileContext(nc) as tc:
        with (
            tc.tile_pool(name="const", bufs=1) as constp,
            tc.tile_pool(name="io", bufs=2) as iop,
            tc.tile_pool(name="g", bufs=2) as gp,
            tc.tile_pool(name="mw", bufs=3) as mwp,
            tc.tile_pool(name="small", bufs=2) as smp,
            tc.tile_pool(name="na", bufs=2) as nap,
            tc.tile_pool(name="psA", bufs=2, space="PSUM") as psA,
            tc.tile_pool(name="psG", bufs=2, space="PSUM") as psG,
            tc.tile_pool(name="psT", bufs=2, space="PSUM") as psT,
        ):
            # ---- persistent constants ----
            ident = constp.tile([128, 128], bf)
            from concourse.masks import make_identity

            make_identity(nc, ident[:])
            w1sb = constp.tile([128, 2 * 512], bf)
            for k in range(2):
                nc.sync.dma_start(
                    out=w1sb[:, k * 512 : (k + 1) * 512],
                    in_=w1_in[k * 128 : (k + 1) * 128, :],
                )
            w2esb = constp.tile([128, 4 * 258], bf)
            for k in range(4):
                nc.sync.dma_start(
                    out=w2esb[:, k * 258 : (k + 1) * 258],
                    in_=w2e_in[k * 128 : (k + 1) * 128, :],
                )
            w3esb = constp.tile([128, 2 * 42], bf)
            for k in range(2):
                nc.sync.dma_start(
                    out=w3esb[:, k * 42 : (k + 1) * 42],
                    in_=w3e_in[k * 128 : (k + 1) * 128, :],
                )
            p_sb = [
                constp.tile([128, 2 * T], bf, tag=f"p{l}", name=f"p_sb{l}")
                for l in range(3)
            ]
            nc.sync.dma_start(out=p_sb[0][:], in_=p1_in[:])
            kc_reg = nc.gpsimd.to_reg(Kc)

            for layer in range(3):
                elem = cfg.ELEM[layer]
                F = cfg.FAGG[layer]
                tbl = tables[layer]
                for t in range(T):
                    rows = min(P, cfg.ND - t * P)
                    # ---- streams ----
                    idx_t = iop.tile([128, 2 * W], i16, tag="idx")
                    nc.sync.dma_start(out=idx_t[:], in_=idxs_in[t])
                    sel_t = iop.tile([128, CH * 128], bf, tag="sel")
                    nc.sync.dma_start(out=sel_t[:], in_=sel_in[t])
                    selt_t = iop.tile([128, CH * 128], bf, tag="selt")
                    nc.sync.dma_start(out=selt_t[:], in_=selt_in[t])

                    # ---- gathers (one per src half) ----
                    g_t = gp.tile([128, CH * elem], bf, tag="g")
                    g3 = g_t[:].rearrange("p (c e) -> p c e", e=elem)
                    for h in range(2):
                        nc.gpsimd.dma_gather(
                            out_ap=g3[:, h * K_half : (h + 1) * K_half, :],
                            in_ap=tbl[h * cfg.HALF : (h + 1) * cfg.HALF, :],
                            idxs_ap=idx_t[:, h * W : (h + 1) * W],
                            num_idxs=Kc,
                            num_idxs_reg=kc_reg,
                            elem_size=elem,
                            single_packet=False,
                            queue_num=(2 * t + h) % 4,
                        )

                    # ---- per-edge p[dst]/ph[dst] via SelT matmuls ----
                    pl_ps = psA.tile([128, 2 * CH], f32, tag="pl")
                    for k in range(CH):
                        nc.tensor.matmul(
                            pl_ps[:, 2 * k : 2 * k + 2],
                            lhsT=selt_t[:, k * 128 : (k + 1) * 128],
                            rhs=p_sb[layer][:, 2 * t : 2 * t + 2],
                            start=True,
                            stop=True,
                        )
                    pl3 = pl_ps[:].rearrange("p (c two) -> p c two", two=2)

                    # ---- per-edge weights w = max(u1*p, u1h*ph) ----
                    # (u1,u1h)x(p,ph) pairs in one TT per half, then a
                    # max-reduce over the pair dim.  Split per src-half so
                    # each op waits on only one gather DMA (2-wait ISA limit).
                    t12 = smp.tile([128, 2 * CH], f32, tag="t12")
                    t123 = t12[:].rearrange("p (c two) -> p c two", two=2)
                    w_t = smp.tile([128, CH], f32, tag="w")
                    for h in range(2):
                        hs = slice(h * K_half, (h + 1) * K_half)
                        nc.vector.tensor_tensor(
                            out=t123[:, hs, :],
                            in0=g3[:, hs, F : F + 2],
                            in1=pl3[:, hs, :],
                            op=ALU.mult,
                        )
                    nc.vector.reduce_max(out=w_t[:], in_=t123[:, :, :], axis=AX.X)

                    # ---- weighted aggregation into PSUM ----
                    mw_all = mwp.tile([128, CH * 128], bf, tag="mw")
                    w_b = (
                        w_t[:]
                        .rearrange("p (c o) -> p c o", o=1)
                        .to_broadcast([128, CH, 128])
                    )
                    nc.vector.tensor_tensor(
                        out=mw_all[:].rearrange("p (c d) -> p c d", d=128),
                        in0=sel_t[:].rearrange("p (c d) -> p c d", d=128),
                        in1=w_b,
                        op=ALU.mult,
                    )
                    agg_ps = psA.tile([128, F + 3], f32, tag="agg")
                    for k in range(CH):
                        nc.tensor.matmul(
                            agg_ps[:],
                            lhsT=mw_all[:, k * 128 : (k + 1) * 128],
                            rhs=g3[:, k, 0 : F + 3],
                            start=(k == 0),
                            stop=(k == CH - 1),
                        )

                    # ---- normalize: out = agg / (den + 1e-9) ----
                    dtmp = smp.tile([128, 1], f32, tag="dtmp")
                    dinv = smp.tile([128, 1], f32, tag="dinv")
                    nc.vector.tensor_scalar(
                        out=dtmp[:],
                        in0=agg_ps[:, F + 2 : F + 3],
                        scalar1=1e-9,
                        scalar2=None,
                        op0=ALU.add,
                    )
                    nc.vector.reciprocal(dinv[:], dtmp[:])

                    dinv_b = (
                        dinv[:]
                        .rearrange("p (c o) -> p c o", o=1)
                        .to_broadcast([128, 1, 256])[:, 0, :]
                    )
                    if layer == 0:
                        na = nap.tile([128, 256], bf, tag="na")
                        nc.vector.tensor_tensor(
                            out=na[:], in0=agg_ps[:, 0:256], in1=dinv_b, op=ALU.mult
                        )
                        # transpose -> GEMM W1 -> relu -> transpose -> GEMM W2e
                        naT = nap.tile([128, 256], bf, tag="naT")
                        for fb in range(2):
                            trp = psT.tile([128, 128], bf, tag="tr")
                            nc.tensor.transpose(
                                trp[:], na[:, fb * 128 : (fb + 1) * 128], ident[:]
                            )
                            nc.scalar.copy(
                                naT[:, fb * 128 : (fb + 1) * 128], trp[:]
                            )
                        o1_ps = psG.tile([128, 512], f32, tag="gemm")
                        for k in range(2):
                            nc.tensor.matmul(
                                o1_ps[:],
                                lhsT=naT[:, k * 128 : (k + 1) * 128],
                                rhs=w1sb[:, k * 512 : (k + 1) * 512],
                                start=(k == 0),
                                stop=(k == 1),
                            )
                        r1 = nap.tile([128, 512], bf, tag="r1")
                        nc.scalar.activation(r1[:], o1_ps[:], AF.Relu)
                        r1T = nap.tile([128, 512], bf, tag="r1T")
                        for fb in range(4):
                            trp = psT.tile([128, 128], bf, tag="tr")
                            nc.tensor.transpose(
                                trp[:], r1[:, fb * 128 : (fb + 1) * 128], ident[:]
                            )
                            nc.scalar.copy(
                                r1T[:, fb * 128 : (fb + 1) * 128], trp[:]
                            )
                        h2_ps = psG.tile([128, 512], f32, tag="gemm", name="h2_ps")[:, 0:258]
                        for k in range(4):
                            nc.tensor.matmul(
                                h2_ps[:],
                                lhsT=r1T[:, k * 128 : (k + 1) * 128],
                                rhs=w2esb[:, k * 258 : (k + 1) * 258],
                                start=(k == 0),
                                stop=(k == 3),
                            )
                        # table2 block: [h2 | u1 | u1h | 1]
                        blk = nap.tile([128, 259], bf, tag="blk")
                        nc.scalar.copy(blk[:, 0:256], h2_ps[:, 0:256])
                        nc.scalar.activation(
                            blk[:, 256:257], h2_ps[:, 256:257], AF.Exp
                        )
                        nc.scalar.activation(
                            blk[:, 257:258], h2_ps[:, 256:257], AF.Exp, scale=0.2
                        )
                        nc.vector.memset(blk[:, 258:259], 1.0)
                        nc.scalar.activation(
                            p_sb[1][:, 2 * t : 2 * t + 1], h2_ps[:, 257:258], AF.Exp
                        )
                        nc.scalar.activation(
                            p_sb[1][:, 2 * t + 1 : 2 * t + 2],
                            h2_ps[:, 257:258],
                            AF.Exp,
                            scale=0.2,
                        )
                        nc.sync.dma_start(
                            out=agin2[t * P : t * P + rows, 0:259],
                            in_=blk[:rows, :],
                        )
                    elif layer == 1:
                        na2 = nap.tile([128, 256], f32, tag="na2")
                        nc.vector.tensor_tensor(
                            out=na2[:], in0=agg_ps[:, 0:256], in1=dinv_b, op=ALU.mult
                        )
                        r2 = nap.tile([128, 256], bf, tag="na")
                        nc.scalar.activation(r2[:], na2[:], AF.Relu)
                        r2T = nap.tile([128, 256], bf, tag="naT")
                        for fb in range(2):
                            trp = psT.tile([128, 128], bf, tag="tr")
                            nc.tensor.transpose(
                                trp[:], r2[:, fb * 128 : (fb + 1) * 128], ident[:]
                            )
                            nc.scalar.copy(
                                r2T[:, fb * 128 : (fb + 1) * 128], trp[:]
                            )
                        h3_ps = psG.tile([128, 512], f32, tag="gemm", name="h3_ps")[:, 0:42]
                        for k in range(2):
                            nc.tensor.matmul(
                                h3_ps[:],
                                lhsT=r2T[:, k * 128 : (k + 1) * 128],
                                rhs=w3esb[:, k * 42 : (k + 1) * 42],
                                start=(k == 0),
                                stop=(k == 1),
                            )
                        blk = nap.tile([128, 43], bf, tag="blk3")
                        nc.scalar.copy(blk[:, 0:40], h3_ps[:, 0:40])
                        nc.scalar.activation(blk[:, 40:41], h3_ps[:, 40:41], AF.Exp)
                        nc.scalar.activation(
                            blk[:, 41:42], h3_ps[:, 40:41], AF.Exp, scale=0.2
                        )
                        nc.vector.memset(blk[:, 42:43], 1.0)
                        nc.scalar.activation(
                            p_sb[2][:, 2 * t : 2 * t + 1], h3_ps[:, 41:42], AF.Exp
                        )
                        nc.scalar.activation(
                            p_sb[2][:, 2 * t + 1 : 2 * t + 2],
                            h3_ps[:, 41:42],
                            AF.Exp,
                            scale=0.2,
                        )
                        nc.sync.dma_start(
                            out=agin3[t * P : t * P + rows, 0:43],
                            in_=blk[:rows, :],
                        )
                    else:
                        # softmax over the 40 classes
                        o3 = nap.tile([128, 40], f32, tag="o3")
                        nc.vector.tensor_tensor(
                            out=o3[:],
                            in0=agg_ps[:, 0:40],
                            in1=dinv_b[:, 0:40],
                            op=ALU.mult,
                        )
                        m = smp.tile([128, 1], f32, tag="m")
                        nc.vector.reduce_max(out=m[:], in_=o3[:], axis=AX.X)
                        negm = smp.tile([128, 1], f32, tag="negm")
                        nc.vector.tensor_scalar(
                            out=negm[:],
                            in0=m[:],
                            scalar1=-1.0,
                            scalar2=None,
                            op0=ALU.mult,
                        )
                        e_t = nap.tile([128, 40], f32, tag="et")
                        nc.scalar.activation(
                            e_t[:], o3[:], AF.Exp, bias=negm[:, 0:1]
                        )
                        s = smp.tile([128, 1], f32, tag="s")
                        nc.vector.reduce_sum(out=s[:], in_=e_t[:], axis=AX.X)
                        sinv = smp.tile([128, 1], f32, tag="sinv")
                        nc.vector.reciprocal(sinv[:], s[:])
                        fin = nap.tile([128, 40], f32, tag="fin")
                        sinv_b = (
                            sinv[:]
                            .rearrange("p (c o) -> p c o", o=1)
                            .to_broadcast([128, 1, 40])[:, 0, :]
                        )
                        nc.vector.tensor_tensor(
                            out=fin[:], in0=e_t[:], in1=sinv_b, op=ALU.mult
                        )
                        nc.sync.dma_start(
                            out=out_d[t * P : t * P + rows, :], in_=fin[:rows, :]
                        )

                if layer == 0:
                    nc.gpsimd.collective_compute(
                        "AllGather",
                        ALU.bypass,
                        replica_groups=[list(range(C))],
                        ins=[agin2[:]],
                        outs=[table2[:]],
                    )
                elif layer == 1:
                    nc.gpsimd.collective_compute(
                        "AllGather",
                        ALU.bypass,
                        replica_groups=[list(range(C))],
                        ins=[agin3[:]],
                        outs=[table3[:]],
                    )
    nc.finalize()  # Bacc.compile(): wait-count legalization etc.
    return nc


def kernel(**inputs) -> np.ndarray:
    in_maps, cfg = preprocess(**inputs)
    nc = build_program(cfg)
    res = run_bass_kernel_spmd(nc, in_maps, core_ids=list(range(cfg.C)))
    outs = [res.results[c]["out"] for c in range(cfg.C)]
    return np.concatenate(outs, axis=0).astype(np.float32)


if __name__ == "__main__":
    import jax

    jax.config.update("jax_platforms", "cpu")
    import reference

    inputs = {k: np.asarray(v) for k, v in reference.setup_inputs().items()}
    out = kernel(**inputs)
    print("kernel output", out.shape, out.dtype)


# revision 14
# speedup vs baseline: 1.4321x; 1.0173x over previous
"""Trainium2 Bass kernel for a 3-layer GAT (nn_GAT_30030411334390).

Strategy
--------
* Shard by destination node range: core c owns dst nodes [c*6250, (c+1)*6250).
  Each core aggregates messages for its own dst nodes only -> no reduce
  collectives are needed, just an AllGather of the per-node feature table
  between layers.
* Attention math is restructured so no per-edge transcendentals are needed:
      exp(leakyrelu(s1+s2)) = max(e^(s1+s2), e^(0.2(s1+s2)))
                            = max(u1[src]*p[dst], u1h[src]*ph[dst])
  with u1 = exp(s1), u1h = exp(0.2*s1) per node (and p/ph from s2).
  The segment-max subtraction of the reference softmax cancels exactly.
* Layer 1 aggregates x (256 wide) BEFORE the GEMM (linearity); layers 2/3
  aggregate post-GEMM features (256 / 40 wide).
* Per dst-tile of 128 nodes, all in-edges are gathered with one dma_gather
  per src half-table (int16 index limit), rows [feat | u1 | u1h | 1 | pad]
  in bf16.  Edge->dst selection matrices (static, graph-only) are built on
  the host and streamed; each 128-edge chunk costs one DVE op (weight
  scaling) and one accumulating PE matmul.
"""

import os
import sys

import numpy as np
import ml_dtypes

sys.path.insert(0, "/opt/trn_rl_repo")

import concourse.bass as bass
from concourse import bacc
import concourse.mybir as mybir
import concourse.tile as tile
from concourse.bass_utils import run_bass_kernel_spmd

BF16 = ml_dtypes.bfloat16
AF = mybir.ActivationFunctionType
ALU = mybir.AluOpType
AX = mybir.AxisListType


class Cfg:
    N = 50000          # nodes
    E = 800000         # edges
    C = 8              # cores
    P = 128
    FIN = 256          # x width
    F1 = 512           # layer-1 GEMM output width
    F2 = 256           # layer-2 feature width
    F3 = 40            # n classes
    ND = N // C        # dst nodes per core
    T = (ND + P - 1) // P      # dst tiles per core
    HALF = 25000       # src half-table size (< 32768 for int16 gather idx)
    # gather-table row sizes in bf16 elements (multiple of 128 for dma_gather)
    ELEM = (384, 384, 128)
    # aggregated feature widths per layer
    FAGG = (256, 256, 40)
    K_half = None      # chunks of 128 edges per (tile, half); set from data


def _wrap_idx(idx_rows: np.ndarray) -> np.ndarray:
    """[G, Kc] int16 -> [G, 128, Kc//16] in dma_gather SBUF layout:
    element j of a gather goes to partition j%16, column j//16, and the
    16-partition pattern is replicated 8x down the 128 partitions."""
    G, Kc = idx_rows.shape
    w = idx_rows.reshape(G, Kc // 16, 16).transpose(0, 2, 1)  # [G,16,W]
    return np.tile(w, (1, 8, 1))  # [G,128,W]


def preprocess(x, edge_idx, W1, a1s, a1d, W2, a2s, a2d, W3, a3s, a3d):
    """Host-side sharding / metadata construction. Returns (in_maps, cfg)."""
    cfg = Cfg()
    N, E, C, P, T = cfg.N, cfg.E, cfg.C, cfg.P, cfg.T
    x = np.asarray(x, dtype=np.float32)
    src = np.asarray(edge_idx[0], dtype=np.int64)
    dst = np.asarray(edge_idx[1], dtype=np.int64)

    # ---- group edges by (core, dst-tile, src-half) ----
    core = dst // cfg.ND
    rel = dst - core * cfg.ND
    tl = rel // P
    ld = rel - tl * P
    half = (src >= cfg.HALF).astype(np.int64)
    gid = ((core * T + tl) * 2 + half).astype(np.int64)
    NG = C * T * 2
    order = np.argsort(gid, kind="stable")
    counts = np.bincount(gid, minlength=NG)
    offs = np.zeros(NG + 1, dtype=np.int64)
    np.cumsum(counts, out=offs[1:])
    pos = np.arange(E, dtype=np.int64) - offs[gid[order]]

    K_half = int((counts.max() + P - 1) // P)
    cfg.K_half = K_half
    Kc = K_half * P
    CH = 2 * K_half

    # ---- int16 gather indices (pads -> 0, masked via zero rows in Sel) ----
    idx_rows = np.zeros((NG, Kc), dtype=np.int16)
    idx_rows[gid[order], pos] = (src[order] - cfg.HALF * half[order]).astype(np.int16)
    idx_wrapped = _wrap_idx(idx_rows).reshape(C, T, 2, 128, Kc // 16)
    # halves side by side on the free dim: [C, T, 128, 2*W]
    idxs = idx_wrapped.transpose(0, 1, 3, 2, 4).reshape(C, T, 128, 2 * (Kc // 16))
    idxs = np.ascontiguousarray(idxs)

    # ---- selection matrices (static, shared across layers) ----
    # Sel[c,t]  [128e, CH*128]: chunk k block has [e, d]=1 iff edge slot e of
    #   chunk k targets local dst d.  Pad slots are zero rows.
    # SelT[c,t] [128d, CH*128]: transposed blocks.
    sel = np.zeros((C, T, 128, CH * 128), dtype=BF16)
    selt = np.zeros((C, T, 128, CH * 128), dtype=BF16)
    e_core = core[order]
    e_tile = tl[order]
    e_ld = ld[order]
    e_half = half[order]
    chunk = e_half * K_half + pos // P
    eslot = pos % P
    sel[e_core, e_tile, eslot, chunk * 128 + e_ld] = 1
    selt[e_core, e_tile, e_ld, chunk * 128 + eslot] = 1

    # ---- layer-1 node scalars (host: tiny matvecs on inputs) ----
    b1s = (np.asarray(W1) @ np.asarray(a1s)).astype(np.float32)
    b1d = (np.asarray(W1) @ np.asarray(a1d)).astype(np.float32)
    s1 = x @ b1s
    s2 = x @ b1d

    table1 = np.zeros((N, cfg.ELEM[0]), dtype=BF16)
    table1[:, : cfg.FIN] = x.astype(BF16)
    table1[:, cfg.FIN] = np.exp(s1).astype(BF16)
    table1[:, cfg.FIN + 1] = np.exp(0.2 * s1).astype(BF16)
    table1[:, cfg.FIN + 2] = 1.0

    # per-core per-dst p/ph for layer 1: [128, 2*T]
    p1 = np.zeros((C, 128, 2 * T), dtype=BF16)
    s2_pad = np.zeros(C * T * P, dtype=np.float32)
    for c in range(C):
        s2_pad[c * T * P : c * T * P + cfg.ND] = s2[c * cfg.ND : (c + 1) * cfg.ND]
    s2g = s2_pad.reshape(C, T, P)
    p1[:, :, 0::2] = np.exp(s2g).transpose(0, 2, 1).astype(BF16)
    p1[:, :, 1::2] = np.exp(0.2 * s2g).transpose(0, 2, 1).astype(BF16)

    # ---- weights (bf16, augmented with attention columns) ----
    w1 = np.asarray(W1, dtype=np.float32).astype(BF16)  # [256, 512]
    w2e = np.concatenate(
        [
            np.asarray(W2, dtype=np.float32),
            (np.asarray(W2) @ np.asarray(a2s))[:, None],
            (np.asarray(W2) @ np.asarray(a2d))[:, None],
        ],
        axis=1,
    ).astype(BF16)  # [512, 258]
    w3e = np.concatenate(
        [
            np.asarray(W3, dtype=np.float32),
            (np.asarray(W3) @ np.asarray(a3s))[:, None],
            (np.asarray(W3) @ np.asarray(a3d))[:, None],
        ],
        axis=1,
    ).astype(BF16)  # [256, 42]

    in_maps = []
    for c in range(C):
        in_maps.append(
            {
                "table1": table1,
                "sel": np.ascontiguousarray(sel[c]),
                "selt": np.ascontiguousarray(selt[c]),
                "idxs": np.ascontiguousarray(idxs[c]),
                "p1": np.ascontiguousarray(p1[c]),
                "w1": w1,
                "w2e": w2e,
                "w3e": w3e,
            }
        )
    return in_maps, cfg


def build_program(cfg):
    N, C, P, T = cfg.N, cfg.C, cfg.P, cfg.T
    K_half = cfg.K_half
    Kc = K_half * P
    CH = 2 * K_half
    W = Kc // 16
    bf = mybir.dt.bfloat16
    f32 = mybir.dt.float32
    i16 = mybir.dt.int16

    nc = bacc.Bacc("TRN2", num_devices=C, num_swdge_queues=4)

    table1 = nc.dram_tensor("table1", [N, cfg.ELEM[0]], bf, kind="ExternalInput")
    sel_in = nc.dram_tensor("sel", [T, 128, CH * 128], bf, kind="ExternalInput")
    selt_in = nc.dram_tensor("selt", [T, 128, CH * 128], bf, kind="ExternalInput")
    idxs_in = nc.dram_tensor("idxs", [T, 128, 2 * W], i16, kind="ExternalInput")
    p1_in = nc.dram_tensor("p1", [128, 2 * T], bf, kind="ExternalInput")
    w1_in = nc.dram_tensor("w1", [256, 512], bf, kind="ExternalInput")
    w2e_in = nc.dram_tensor("w2e", [512, 258], bf, kind="ExternalInput")
    w3e_in = nc.dram_tensor("w3e", [256, 42], bf, kind="ExternalInput")
    out_d = nc.dram_tensor("out", [cfg.ND, cfg.F3], f32, kind="ExternalOutput")

    agin2 = nc.dram_tensor("agin2", [cfg.ND, cfg.ELEM[1]], bf)
    table2 = nc.dram_tensor("table2", [N, cfg.ELEM[1]], bf, addr_space="Shared")
    agin3 = nc.dram_tensor("agin3", [cfg.ND, cfg.ELEM[2]], bf)
    table3 = nc.dram_tensor("table3", [N, cfg.ELEM[2]], bf, addr_space="Shared")
    tables = (table1, table2, table3)

    with tile.TileContext(nc) as tc:
        with (
            tc.tile_pool(name="const", bufs=1) as constp,
            tc.tile_pool(name="io", bufs=4) as iop,
            tc.tile_pool(name="g", bufs=3) as gp,
            tc.tile_pool(name="mw", bufs=3) as mwp,
            tc.tile_pool(name="small", bufs=2) as smp,
            tc.tile_pool(name="na", bufs=2) as nap,
            tc.tile_pool(name="psA", bufs=2, space="PSUM") as psA,
            tc.tile_pool(name="psG", bufs=2, space="PSUM") as psG,
            tc.tile_pool(name="psT", bufs=2, space="PSUM") as psT,
        ):
            # ---- persistent constants ----
            ident = constp.tile([128, 128], bf)
            from concourse.masks import make_identity

            make_identity(nc, ident[:])
            w1sb = constp.tile([128, 2 * 512], bf)
            for k in range(2):
                nc.sync.dma_start(
                    out=w1sb[:, k * 512 : (k + 1) * 512],
                    in_=w1_in[k * 128 : (k + 1) * 128, :],
                )
            w2esb = constp.tile([128, 4 * 258], bf)
            for k in range(4):
                nc.sync.dma_start(
                    out=w2esb[:, k * 258 : (k + 1) * 258],
                    in_=w2e_in[k * 128 : (k + 1) * 128, :],
                )
            w3esb = constp.tile([128, 2 * 42], bf)
            for k in range(2):
                nc.sync.dma_start(
                    out=w3esb[:, k * 42 : (k + 1) * 42],
                    in_=w3e_in[k * 128 : (k + 1) * 128, :],
                )
            p_sb = [
                constp.tile([128, 2 * T], bf, tag=f"p{l}", name=f"p_sb{l}")
                for l in range(3)
            ]
            nc.sync.dma_start(out=p_sb[0][:], in_=p1_in[:])
            kc_reg = nc.gpsimd.to_reg(Kc)
            idx_all = constp.tile([128, T * 2 * W], i16, name="idx_all")
            nc.sync.dma_start(
                out=idx_all[:].rearrange("p (t w) -> p t w", w=2 * W),
                in_=idxs_in[:, :, :].rearrange("t p w -> p t w"),
            )

            for layer in range(3):
                elem = cfg.ELEM[layer]
                F = cfg.FAGG[layer]
                tbl = tables[layer]
                for t in range(T):
                    rows = min(P, cfg.ND - t * P)
                    # ---- streams ----
                    idx_t = idx_all[:, t * 2 * W : (t + 1) * 2 * W]
                    sel_t = iop.tile([128, CH * 128], bf, tag="sel")
                    nc.scalar.dma_start(out=sel_t[:], in_=sel_in[t])
                    selt_t = iop.tile([128, CH * 128], bf, tag="selt")
                    nc.scalar.dma_start(out=selt_t[:], in_=selt_in[t])

                    # ---- gathers (one per src half) ----
                    g_t = gp.tile([128, CH * elem], bf, tag="g")
                    g3 = g_t[:].rearrange("p (c e) -> p c e", e=elem)
                    for h in range(2):
                        nc.gpsimd.dma_gather(
                            out_ap=g3[:, h * K_half : (h + 1) * K_half, :],
                            in_ap=tbl[h * cfg.HALF : (h + 1) * cfg.HALF, :],
                            idxs_ap=idx_t[:, h * W : (h + 1) * W],
                            num_idxs=Kc,
                            num_idxs_reg=kc_reg,
                            elem_size=elem,
                            single_packet=False,
                            queue_num=(2 * t + h) % 4,
                        )

                    # ---- per-edge p[dst]/ph[dst] via SelT matmuls ----
                    pl_ps = psA.tile([128, 2 * CH], f32, tag="pl")
                    for k in range(CH):
                        nc.tensor.matmul(
                            pl_ps[:, 2 * k : 2 * k + 2],
                            lhsT=selt_t[:, k * 128 : (k + 1) * 128],
                            rhs=p_sb[layer][:, 2 * t : 2 * t + 2],
                            start=True,
                            stop=True,
                        )
                    pl3 = pl_ps[:].rearrange("p (c two) -> p c two", two=2)

                    # ---- per-edge weights w = max(u1*p, u1h*ph) ----
                    # (u1,u1h)x(p,ph) pairs in one TT per half, then a
                    # max-reduce over the pair dim.  Split per src-half so
                    # each op waits on only one gather DMA (2-wait ISA limit).
                    t12 = smp.tile([128, 2 * CH], f32, tag="t12")
                    t123 = t12[:].rearrange("p (c two) -> p c two", two=2)
                    w_t = smp.tile([128, CH], f32, tag="w")
                    for h in range(2):
                        hs = slice(h * K_half, (h + 1) * K_half)
                        nc.vector.tensor_tensor(
                            out=t123[:, hs, :],
                            in0=g3[:, hs, F : F + 2],
                            in1=pl3[:, hs, :],
                            op=ALU.mult,
                        )
                    nc.vector.reduce_max(out=w_t[:], in_=t123[:, :, :], axis=AX.X)

                    # ---- weighted aggregation into PSUM ----
                    mw_all = mwp.tile([128, CH * 128], bf, tag="mw")
                    w_b = (
                        w_t[:]
                        .rearrange("p (c o) -> p c o", o=1)
                        .to_broadcast([128, CH, 128])
                    )
                    nc.vector.tensor_tensor(
                        out=mw_all[:].rearrange("p (c d) -> p c d", d=128),
                        in0=sel_t[:].rearrange("p (c d) -> p c d", d=128),
                        in1=w_b,
                        op=ALU.mult,
                    )
                    agg_ps = psA.tile([128, F + 3], f32, tag="agg")
                    for k in range(CH):
                        nc.tensor.matmul(
                            agg_ps[:],
                            lhsT=mw_all[:, k * 128 : (k + 1) * 128],
                            rhs=g3[:, k, 0 : F + 3],
                            start=(k == 0),
                            stop=(k == CH - 1),
                        )

                    # ---- normalize: out = agg / (den + 1e-9) ----
                    dtmp = smp.tile([128, 1], f32, tag="dtmp")
                    dinv = smp.tile([128, 1], f32, tag="dinv")
                    nc.vector.tensor_scalar(
                        out=dtmp[:],
                        in0=agg_ps[:, F + 2 : F + 3],
                        scalar1=1e-9,
                        scalar2=None,
                        op0=ALU.add,
                    )
                    nc.vector.reciprocal(dinv[:], dtmp[:])

                    dinv_b = (
                        dinv[:]
                        .rearrange("p (c o) -> p c o", o=1)
                        .to_broadcast([128, 1, 256])[:, 0, :]
                    )
                    if layer == 0:
                        na = nap.tile([128, 256], bf, tag="na")
                        nc.vector.tensor_tensor(
                            out=na[:], in0=agg_ps[:, 0:256], in1=dinv_b, op=ALU.mult
                        )
                        # transpose -> GEMM W1 -> relu -> transpose -> GEMM W2e
                        naT = nap.tile([128, 256], bf, tag="naT")
                        for fb in range(2):
                            trp = psT.tile([128, 128], bf, tag="tr")
                            nc.tensor.transpose(
                                trp[:], na[:, fb * 128 : (fb + 1) * 128], ident[:]
                            )
                            nc.scalar.copy(
                                naT[:, fb * 128 : (fb + 1) * 128], trp[:]
                            )
                        o1_ps = psG.tile([128, 512], f32, tag="gemm")
                        for k in range(2):
                            nc.tensor.matmul(
                                o1_ps[:],
                                lhsT=naT[:, k * 128 : (k + 1) * 128],
                                rhs=w1sb[:, k * 512 : (k + 1) * 512],
                                start=(k == 0),
                                stop=(k == 1),
                            )
                        r1 = nap.tile([128, 512], bf, tag="r1")
                        nc.scalar.activation(r1[:], o1_ps[:], AF.Relu)
                        r1T = nap.tile([128, 512], bf, tag="r1T")
                        for fb in range(4):
                            trp = psT.tile([128, 128], bf, tag="tr")
                            nc.tensor.transpose(
                                trp[:], r1[:, fb * 128 : (fb + 1) * 128], ident[:]
                            )
                            nc.scalar.copy(
                                r1T[:, fb * 128 : (fb + 1) * 128], trp[:]
                            )
                        h2_ps = psG.tile([128, 512], f32, tag="gemm", name="h2_ps")[:, 0:258]
                        for k in range(4):
                            nc.tensor.matmul(
                                h2_ps[:],
                                lhsT=r1T[:, k * 128 : (k + 1) * 128],
                                rhs=w2esb[:, k * 258 : (k + 1) * 258],
                                start=(k == 0),
                                stop=(k == 3),
                            )
                        # table2 block: [h2 | u1 | u1h | 1]
                        blk = nap.tile([128, 259], bf, tag="blk")
                        nc.scalar.copy(blk[:, 0:256], h2_ps[:, 0:256])
                        nc.scalar.activation(
                            blk[:, 256:257], h2_ps[:, 256:257], AF.Exp
                        )
                        nc.scalar.activation(
                            blk[:, 257:258], h2_ps[:, 256:257], AF.Exp, scale=0.2
                        )
                        nc.vector.memset(blk[:, 258:259], 1.0)
                        nc.scalar.activation(
                            p_sb[1][:, 2 * t : 2 * t + 1], h2_ps[:, 257:258], AF.Exp
                        )
                        nc.scalar.activation(
                            p_sb[1][:, 2 * t + 1 : 2 * t + 2],
                            h2_ps[:, 257:258],
                            AF.Exp,
                            scale=0.2,
                        )
                        nc.sync.dma_start(
                            out=agin2[t * P : t * P + rows, 0:259],
                            in_=blk[:rows, :],
                        )
                    elif layer == 1:
                        na2 = nap.tile([128, 256], f32, tag="na2")
                        nc.vector.tensor_tensor(
                            out=na2[:], in0=agg_ps[:, 0:256], in1=dinv_b, op=ALU.mult
                        )
                        r2 = nap.tile([128, 256], bf, tag="na")
                        nc.scalar.activation(r2[:], na2[:], AF.Relu)
                        r2T = nap.tile([128, 256], bf, tag="naT")
                        for fb in range(2):
                            trp = psT.tile([128, 128], bf, tag="tr")
                            nc.tensor.transpose(
                                trp[:], r2[:, fb * 128 : (fb + 1) * 128], ident[:]
                            )
                            nc.scalar.copy(
                                r2T[:, fb * 128 : (fb + 1) * 128], trp[:]
                            )
                        h3_ps = psG.tile([128, 512], f32, tag="gemm", name="h3_ps")[:, 0:42]
                        for k in range(2):
                            nc.tensor.matmul(
                                h3_ps[:],
                                lhsT=r2T[:, k * 128 : (k + 1) * 128],
                                rhs=w3esb[:, k * 42 : (k + 1) * 42],
                                start=(k == 0),
                                stop=(k == 1),
                            )
                        blk = nap.tile([128, 43], bf, tag="blk3")
                        nc.scalar.copy(blk[:, 0:40], h3_ps[:, 0:40])
                        nc.scalar.activation(blk[:, 40:41], h3_ps[:, 40:41], AF.Exp)
                        nc.scalar.activation(
                            blk[:, 41:42], h3_ps[:, 40:41], AF.Exp, scale=0.2
                        )
                        nc.vector.memset(blk[:, 42:43], 1.0)
                        nc.scalar.activation(
                            p_sb[2][:, 2 * t : 2 * t + 1], h3_ps[:, 41:42], AF.Exp
                        )
                        nc.scalar.activation(
                            p_sb[2][:, 2 * t + 1 : 2 * t + 2],
                            h3_ps[:, 41:42],
                            AF.Exp,
                            scale=0.2,
                        )
                        nc.sync.dma_start(
                            out=agin3[t * P : t * P + rows, 0:43],
                            in_=blk[:rows, :],
                        )
                    else:
                        # softmax over the 40 classes
                        o3 = nap.tile([128, 40], f32, tag="o3")
                        nc.vector.tensor_tensor(
                            out=o3[:],
                            in0=agg_ps[:, 0:40],
                            in1=dinv_b[:, 0:40],
                            op=ALU.mult,
                        )
                        m = smp.tile([128, 1], f32, tag="m")
                        nc.vector.reduce_max(out=m[:], in_=o3[:], axis=AX.X)
                        negm = smp.tile([128, 1], f32, tag="negm")
                        nc.vector.tensor_scalar(
                            out=negm[:],
                            in0=m[:],
                            scalar1=-1.0,
                            scalar2=None,
                            op0=ALU.mult,
                        )
                        e_t = nap.tile([128, 40], f32, tag="et")
                        nc.scalar.activation(
                            e_t[:], o3[:], AF.Exp, bias=negm[:, 0:1]
                        )
                        s = smp.tile([128, 1], f32, tag="s")
                        nc.vector.reduce_sum(out=s[:], in_=e_t[:], axis=AX.X)
                        sinv = smp.tile([128, 1], f32, tag="sinv")
                        nc.vector.reciprocal(sinv[:], s[:])
                        fin = nap.tile([128, 40], f32, tag="fin")
                        sinv_b = (
                            sinv[:]
                            .rearrange("p (c o) -> p c o", o=1)
                            .to_broadcast([128, 1, 40])[:, 0, :]
                        )
                        nc.vector.tensor_tensor(
                            out=fin[:], in0=e_t[:], in1=sinv_b, op=ALU.mult
                        )
                        nc.sync.dma_start(
                            out=out_d[t * P : t * P + rows, :], in_=fin[:rows, :]
                        )

                if layer == 0:
                    nc.gpsimd.collective_compute(
                        "AllGather",
                        ALU.bypass,
                        replica_groups=[list(range(C))],
                        ins=[agin2[:]],
                        outs=[table2[:]],
                    )
                elif layer == 1:
                    nc.gpsimd.collective_compute(
                        "AllGather",
                        ALU.bypass,
                        replica_groups=[list(range(C))],
                        ins=[agin3[:]],
                        outs=[table3[:]],
                    )
    nc.finalize()  # Bacc.compile(): wait-count legalization etc.
    return nc


def kernel(**inputs) -> np.ndarray:
    in_maps, cfg = preprocess(**inputs)
    nc = build_program(cfg)
    res = run_bass_kernel_spmd(nc, in_maps, core_ids=list(range(cfg.C)))
    outs = [res.results[c]["out"] for c in range(cfg.C)]
    return np.concatenate(outs, axis=0).astype(np.float32)


if __name__ == "__main__":
    import jax

    jax.config.update("jax_platforms", "cpu")
    import reference

    inputs = {k: np.asarray(v) for k, v in reference.setup_inputs().items()}
    out = kernel(**inputs)
    print("kernel output", out.shape, out.dtype)
